# revision 55
# baseline (speedup 1.0000x reference)
"""AdaptiveGraphAttention Trainium2 kernel (8 NeuronCores, data-parallel).

Math: in the reference, logits[b,h,i,j] = a_q[b,h,i] + a_k[b,h,j] +
e_j[b,h,j]*adj[i,j] + attn_b with adj[:,0]=0, adj[:,1:]=1 — the mask and the
j-dependent terms are identical for every query row i, and the a_q/bias terms
are constant over j.  Softmax is shift-invariant, so the attention
distribution p[b,h,:] = softmax_{j>=1}(a_k + e_j) is the same for all i: the
attention matrix is rank-1 and the output is one row per batch, broadcast
over the 256 query positions.  bq/bk/attn_b cancel exactly; bv survives as
an additive constant (sum_j p_j = 1); bv and bo are folded on the host.

Per-head dots fold into small matrices:
  a_k[b,j,h] = nv[b,j,:] @ Uk[:,h],  Uk[d,h] = sum_m Wk[h*64+m, d] * w_k[m]
  e_j[b,j,h] = desc[b,j-1,:] @ Ue[:,h], Ue[h*64+m, h] = w_e(m) (else 0)

Device work per core (4 batches); fp8e4m3 logits path (inputs scaled by
USCALE to clear the fp8 subnormal floor, undone by the exp's scale), bf16
everywhere else, f32 PSUM accumulation:
  c[h,j]    = Uk.T @ nvT[:, j] + Ue.T @ descT[:, j-1]  (PE DoubleRow fp8)
  p[h,:]    = softmax_j(c)   (no max-subtraction: logits are O(1));
              the 1/sum normalization is folded into the p-transpose by
              multiplying with diag(recip) on the PE.
  nvbarT    = nv.T @ pT           [1024, 16] per batch      (PE, one PSUM
              tile for all 8 d-chunks -> single copy to SBUF)
  VbarT     = WvT.T @ nvbarT_all  [1024, 4*16] per d'-chunk (PE)
  ctxT      = blockdiag-select(VbarT)        [1024, 4]      (DVE)
  out       = ctxT.T @ WoT        [4, 1024]                 (PE) -> DMA
bv is folded into the host-side output bias (out += Wo @ bv + bo, exact
since sum_j p_j = 1).

Schedule: the kernel is HBM-DMA-bound (~8.4MB/core at ~358GB/s).  All DMAs
issue up-front on the sync queue in consumption order: per-batch activations
first (batch 0's xT split so the first DR matmul starts ~1us after the first
bytes land), then WvT in cm-halves, then WoT in output-column halves — the
Vbar and out stages chase the incoming weight stream instead of running as a
serial tail after it.

All DRAM inputs are host-prepermuted to [128, chunk, inner] so each DMA
partition row is one contiguous run (descriptor-count relief).
"""

import numpy as np
import ml_dtypes
from contextlib import ExitStack

import concourse.bass as bass
import concourse.mybir as mybir
import concourse.tile as tile
from concourse import bacc
from concourse.bass_utils import run_bass_kernel_spmd
from concourse.masks import make_identity

B, S, D, H, HD = 32, 256, 1024, 16, 64
NCORES = 8
BPC = B // NCORES  # 4 batches per core
F32 = mybir.dt.float32
BF16 = mybir.dt.bfloat16
NPBF = ml_dtypes.bfloat16
F8 = mybir.dt.float8e4
NPF8 = ml_dtypes.float8_e4m3
USCALE = 512.0  # fp8 range lift for the tiny folded U entries
DC = D // 128  # 8 chunks of the model dim
JC = S // 128  # 2 chunks of the sequence dim

_cache = {}


def _build():
    nc = bacc.Bacc("TRN2", target_bir_lowering=False, debug=False,
                   num_devices=NCORES)

    nv_ext = nc.declare_dram_parameter("nv", [BPC, 128, JC, D], BF16,
                                       isOutput=False)
    xt_ext = nc.declare_dram_parameter("xT", [BPC, 128, DC, 2 * S], F8,
                                       isOutput=False)
    u_ext = nc.declare_dram_parameter("U", [128, DC, 2 * H], F8,
                                      isOutput=False)
    wvt_ext = nc.declare_dram_parameter("WvT", [128, DC, DC, 128], BF16,
                                        isOutput=False)
    wot_ext = nc.declare_dram_parameter("WoT", [128, DC, DC, 128], BF16,
                                        isOutput=False)
    out_ext = nc.declare_dram_parameter("out", [128, DC, BPC], F32,
                                        isOutput=True)

    with tile.TileContext(nc) as tc, ExitStack() as ctx:
        wpool = ctx.enter_context(tc.tile_pool(name="w", bufs=1))
        xpool = ctx.enter_context(tc.tile_pool(name="x", bufs=4))
        smpool = ctx.enter_context(tc.tile_pool(name="sm", bufs=2))
        pspool = ctx.enter_context(tc.tile_pool(name="ps", bufs=2,
                                                space="PSUM"))

        # --- resident constants -------------------------------------------
        ones128 = wpool.tile([128, 1], BF16)
        nc.gpsimd.memset(ones128[:], 1.0)
        ones1 = wpool.tile([1, 128], BF16)
        nc.gpsimd.memset(ones1[:], 1.0)
        recips = wpool.tile([1, BPC * H], BF16)
        # double-buffered unnormalized-pT tiles.  Row j=0 (CLS, masked out)
        # needs no masking op: the xT layout plants a -448*sign(w_e) column
        # right before descT, so the e-term drives c[0,h] to ~-2e5 for every
        # head and exp underflows to exactly 0.
        ptus = [wpool.tile([128, JC, H], BF16, name=f"ptu{par}")
                for par in range(2)]
        u_sb = wpool.tile([128, DC, 2 * H], F8)
        # U goes on the gpsimd queue so it doesn't take a slot ahead of the
        # batch-0 activations on the sync queue.
        nc.gpsimd.dma_start(out=u_sb[:], in_=u_ext.ap())

        nvall = wpool.tile([128, DC, BPC * H], BF16)  # nvbarT, all batches
        ctx_sb = wpool.tile([128, DC, BPC], BF16)
        wvt_sb = wpool.tile([128, DC, DC, 128], BF16)
        wot_sb = wpool.tile([128, DC, DC, 128], BF16)

        # --- all input DMAs up-front, in consumption order ----------------
        xt_tiles, nv_tiles = [], []
        for b in range(BPC):
            xt_sb = xpool.tile([128, DC, 2 * S], F8, tag="xt")
            if b == 0:
                nc.sync.dma_start(out=xt_sb[:, 0:2], in_=xt_ext[b, :, 0:2])
                nc.sync.dma_start(out=xt_sb[:, 2:DC], in_=xt_ext[b, :, 2:DC])
            else:
                nc.sync.dma_start(out=xt_sb[:], in_=xt_ext[b])
            nv_sb = xpool.tile([128, JC, D], BF16, tag="nv")
            nc.sync.dma_start(out=nv_sb[:], in_=nv_ext[b])
            xt_tiles.append(xt_sb)
            nv_tiles.append(nv_sb)
        for sl in (slice(0, 4), slice(4, 8)):
            nc.sync.dma_start(out=wvt_sb[:, sl], in_=wvt_ext[:, sl])
        for sl in (slice(0, 4), slice(4, 8)):
            nc.sync.dma_start(out=wot_sb[:, sl], in_=wot_ext[:, sl])

        # --- batch loop, software-pipelined.  Logits come out j-major
        # (cT[j, h], j on partitions) by using xT as the DoubleRow
        # stationary operand — no p-transpose needed, and the softmax's
        # sum/recip/normalize moves OFF the per-batch critical path: nvbarT
        # uses unnormalized exp-weights and the 1/sum is applied to nvall
        # once, after the loop (PSUM f32 keeps the unnormalized sums exact).
        # The e-term needs e_j at row j from descT col j-1: the lhsT slice
        # shifts by one column; for jb=0 the slice starts at the nvT j=255
        # column, which corrupts only row j=0 — zeroed anyway (adj[:,0]=0).
        DR = mybir.MatmulPerfMode.DoubleRow

        def logits(b):
            xt_sb = xt_tiles[b]
            pscs = []
            for jb in range(JC):
                psc = pspool.tile([128, H], F32, tag="s", bufs=4,
                                  name=f"psc{b}_{jb}")
                jcol = jb * 128
                for c2 in range(DC // 2):
                    pair = slice(2 * c2, 2 * c2 + 2)
                    nc.tensor.matmul(psc[:],
                                     xt_sb[:, pair, jcol:jcol + 128],
                                     u_sb[:, pair, 0:H],
                                     start=(c2 == 0), stop=False,
                                     perf_mode=DR)
                for c2 in range(DC // 2):
                    pair = slice(2 * c2, 2 * c2 + 2)
                    nc.tensor.matmul(psc[:],
                                     xt_sb[:, pair,
                                           S + jcol:S + jcol + 128],
                                     u_sb[:, pair, H:2 * H],
                                     start=False, stop=(c2 == DC // 2 - 1),
                                     perf_mode=DR)
                pscs.append(psc)
            return pscs

        def tail(b, pscs):
            nv_sb = nv_tiles[b]
            ptu = ptus[b % 2]
            for jb in range(JC):
                nc.scalar.activation(ptu[:, jb, :], pscs[jb][:],
                                     mybir.ActivationFunctionType.Exp,
                                     scale=1.0 / USCALE)

            # nvbarT (unnormalized) for all 8 d-chunks.  Consecutive cm
            # groups alternate PSUM banks (a new accumulation group in the
            # bank the previous group used stalls the PE ~150ns).
            nb_ps = [pspool.tile([128, DC // 2, H], F32, tag=f"nb{par}",
                                 bufs=1, name=f"nb{par}_{b}")
                     for par in range(2)]
            for cm in range(DC):
                for jc in range(JC):
                    nc.tensor.matmul(nb_ps[cm % 2][:, cm // 2, :],
                                     nv_sb[:, jc, cm * 128:(cm + 1) * 128],
                                     ptu[:, jc, :],
                                     start=(jc == 0), stop=(jc == JC - 1))
            nvb = nvall[:, :, b * H:(b + 1) * H].rearrange(
                "p (c two) h -> p c two h", two=2)
            for par in range(2):
                nc.vector.tensor_copy(nvb[:, :, par, :], nb_ps[par][:])

            # sum_j p and its reciprocal — off the critical path
            s_ps = pspool.tile([1, H], F32, tag="s", bufs=4, name=f"sum{b}")
            for jb in range(JC):
                nc.tensor.matmul(s_ps[:], ones128[:], ptu[:, jb, :],
                                 start=(jb == 0), stop=(jb == JC - 1))
            with nc.allow_low_precision(reason="1/sum in bf16 (~0.4%) is "
                                        "far below the fp8 logits noise"):
                nc.vector.reciprocal(recips[0:1, b * H:(b + 1) * H], s_ps[:])

        pscs_live = logits(0)
        for b in range(BPC):
            pscs_next = logits(b + 1) if b + 1 < BPC else None
            tail(b, pscs_live)
            pscs_live = pscs_next

        # 1/sum broadcast across partitions via PE — consumed by the selects
        # (staged through SBUF: the select already reads Vbar from PSUM)
        r_ps = pspool.tile([128, BPC * H], F32, tag="nb0", bufs=1)
        nc.tensor.matmul(r_ps[:], ones1[:], recips[:], start=True, stop=True)
        r_sb = wpool.tile([128, BPC * H], F32)
        nc.vector.tensor_copy(r_sb[:], r_ps[:])
        for ck in range(DC):
            nc.vector.tensor_mul(nvall[:, ck, :], nvall[:, ck, :], r_sb[:])

        # --- VbarT (unnormalized), blockdiag select fused with the 1/sum
        # scale, and the out-projection's ck-rounds interleaved two chunks
        # behind so OUT finishes right after the last Vbar chunk.
        # Consecutive cm groups alternate PSUM banks. ----------------------
        vb_ps = [pspool.tile([128, DC // 2, BPC * H], F32, tag=f"vb{par}",
                             bufs=1, name=f"vb{par}")
                 for par in range(2)]
        o_ps = [pspool.tile([128, DC // 2, BPC], F32, tag="s", bufs=4,
                            name=f"o{par}")
                for par in range(2)]

        def vbar(cm):
            for ck in range(DC):
                nc.tensor.matmul(vb_ps[cm % 2][:, cm // 2, :],
                                 wvt_sb[:, cm, ck, :],
                                 nvall[:, ck, :],
                                 start=(ck == 0), stop=(ck == DC - 1))

        def select(cm):
            for half in range(2):
                h = 2 * cm + half
                rows = slice(64 * half, 64 * half + 64)
                s_ap = vb_ps[cm % 2][rows, cm // 2, :].rearrange(
                    "p (b h) -> p b h", h=H)[:, :, h]
                nc.vector.tensor_copy(ctx_sb[rows, cm, :], s_ap)

        for cm in range(DC):
            vbar(cm)
            select(cm)
        for ec in range(DC):
            for ck in range(DC):
                nc.tensor.matmul(o_ps[ec % 2][:, ec // 2, :],
                                 wot_sb[:, ck, ec, :],
                                 ctx_sb[:, ck, :],
                                 start=(ck == 0), stop=(ck == DC - 1))

        o_sb = smpool.tile([128, DC, BPC], F32, tag="osb")
        for par in range(2):
            nc.vector.tensor_copy(
                o_sb[:].rearrange("p (e two) b -> p e two b", two=2)
                [:, :, par, :],
                o_ps[par][:])
        nc.sync.dma_start(out=out_ext.ap(), in_=o_sb[:])

    nc.compile()
    return nc


def _prep(desc, nv, Wk, Wv, Wo, attn_w):
    w_k = attn_w[HD:2 * HD]
    w_e = attn_w[2 * HD:]
    Uk = np.einsum('hmd,m->dh', Wk.reshape(H, HD, D), w_k)
    Ue = np.zeros((D, H), np.float32)
    for h in range(H):
        Ue[h * HD:(h + 1) * HD, h] = w_e
    U = np.concatenate([Uk, Ue], axis=1) * USCALE           # [D, 32]
    Up = np.ascontiguousarray(
        U.reshape(DC, 128, 2 * H).swapaxes(0, 1)).astype(NPF8)
    WvTp = np.ascontiguousarray(
        Wv.T.reshape(DC, 128, DC, 128).transpose(1, 2, 0, 3)).astype(NPBF)
    WoTp = np.ascontiguousarray(
        Wo.T.reshape(DC, 128, DC, 128).transpose(1, 0, 2, 3)).astype(NPBF)
    # nv natural, chunked over j: [B, 128, JC, D]
    nvp = np.ascontiguousarray(
        nv.reshape(B, JC, 128, D).swapaxes(1, 2)).astype(NPBF)
    # nv transposed, chunked over d: [B, 128, DC, S]
    nvTp = nv.transpose(0, 2, 1).reshape(B, DC, 128, S).swapaxes(1, 2)
    descTp = desc.transpose(0, 2, 1).reshape(B, DC, 128, S - 1).swapaxes(1, 2)
    # CLS kill column (sits at descT's j=0 slot): -K*sign(w_e) drives the
    # j=0 logit to ~-50 post-scale for every head (w_e is shared across
    # heads), so exp(j=0) ~ 1e-22 — dead, but safe for the Exp table
    kmag = 50.0 / max(np.abs(w_e).sum(), 1e-6)
    neg = np.zeros((D,), np.float32)
    for h in range(H):
        neg[h * HD:(h + 1) * HD] = -kmag * np.sign(w_e)
    negcol = np.broadcast_to(
        neg.reshape(DC, 128).T.reshape(1, 128, DC, 1), (B, 128, DC, 1))
    xTp = np.concatenate([nvTp, negcol, descTp], axis=3).astype(NPF8)
    return Up, WvTp, WoTp, nvp, xTp


def kernel(desc_embeddings, name_value_embeddings, Wq, bq, Wk, bk, Wv, bv,
           attn_w, attn_b, Wo, bo, _trace=False):
    desc = np.asarray(desc_embeddings, np.float32)
    nv = np.asarray(name_value_embeddings, np.float32)
    Up, WvTp, WoTp, nvp, xTp = _prep(
        desc, nv, np.asarray(Wk, np.float32), np.asarray(Wv, np.float32),
        np.asarray(Wo, np.float32), np.asarray(attn_w, np.float32))

    if "nc" not in _cache:
        _cache["nc"] = _build()
    nc = _cache["nc"]

    in_maps = []
    for c in range(NCORES):
        sl = slice(c * BPC, (c + 1) * BPC)
        in_maps.append({
            "nv": np.ascontiguousarray(nvp[sl]),
            "xT": np.ascontiguousarray(xTp[sl]),
            "U": Up, "WvT": WvTp, "WoT": WoTp,
        })
    res = run_bass_kernel_spmd(nc, in_maps, core_ids=list(range(NCORES)),
                               trace=_trace)
    out_rows = np.empty((B, D), np.float32)
    for c in range(NCORES):
        ot = np.asarray(res.results[c]["out"])  # [128, DC, BPC] = outT
        out_rows[c * BPC:(c + 1) * BPC] = ot.transpose(2, 1, 0).reshape(BPC, D)
    bo_eff = (np.asarray(bo, np.float32)
              + np.asarray(Wo, np.float32) @ np.asarray(bv, np.float32))
    out_rows += bo_eff[None, :]
    full = np.broadcast_to(out_rows[:, None, :], (B, S, D))
    if _trace:
        return np.ascontiguousarray(full), res
    return np.ascontiguousarray(full)


# revision 57
# speedup vs baseline: 1.0455x; 1.0455x over previous
"""AdaptiveGraphAttention Trainium2 kernel (8 NeuronCores, data-parallel).

Math: in the reference, logits[b,h,i,j] = a_q[b,h,i] + a_k[b,h,j] +
e_j[b,h,j]*adj[i,j] + attn_b with adj[:,0]=0, adj[:,1:]=1 — the mask and the
j-dependent terms are identical for every query row i, and the a_q/bias terms
are constant over j.  Softmax is shift-invariant, so the attention
distribution p[b,h,:] = softmax_{j>=1}(a_k + e_j) is the same for all i: the
attention matrix is rank-1 and the output is one row per batch, broadcast
over the 256 query positions.  bq/bk/attn_b cancel exactly; bv survives as
an additive constant (sum_j p_j = 1); bv and bo are folded on the host.

Per-head dots fold into small matrices:
  a_k[b,j,h] = nv[b,j,:] @ Uk[:,h],  Uk[d,h] = sum_m Wk[h*64+m, d] * w_k[m]
  e_j[b,j,h] = desc[b,j-1,:] @ Ue[:,h], Ue[h*64+m, h] = w_e(m) (else 0)

Device work per core (4 batches); fp8e4m3 logits path (inputs scaled by
USCALE to clear the fp8 subnormal floor, undone by the exp's scale), bf16
everywhere else, f32 PSUM accumulation:
  c[h,j]    = Uk.T @ nvT[:, j] + Ue.T @ descT[:, j-1]  (PE DoubleRow fp8)
  p[h,:]    = softmax_j(c)   (no max-subtraction: logits are O(1));
              the 1/sum normalization is folded into the p-transpose by
              multiplying with diag(recip) on the PE.
  nvbarT    = nv.T @ pT           [1024, 16] per batch      (PE, one PSUM
              tile for all 8 d-chunks -> single copy to SBUF)
  VbarT     = WvT.T @ nvbarT_all  [1024, 4*16] per d'-chunk (PE)
  ctxT      = blockdiag-select(VbarT)        [1024, 4]      (DVE)
  out       = ctxT.T @ WoT        [4, 1024]                 (PE) -> DMA
bv is folded into the host-side output bias (out += Wo @ bv + bo, exact
since sum_j p_j = 1).

Schedule: the kernel is HBM-DMA-bound (~8.4MB/core at ~358GB/s).  All DMAs
issue up-front on the sync queue in consumption order: per-batch activations
first (batch 0's xT split so the first DR matmul starts ~1us after the first
bytes land), then WvT in cm-halves, then WoT in output-column halves — the
Vbar and out stages chase the incoming weight stream instead of running as a
serial tail after it.

All DRAM inputs are host-prepermuted to [128, chunk, inner] so each DMA
partition row is one contiguous run (descriptor-count relief).
"""

import numpy as np
import ml_dtypes
from contextlib import ExitStack

import concourse.bass as bass
import concourse.mybir as mybir
import concourse.tile as tile
from concourse import bacc
from concourse.bass_utils import run_bass_kernel_spmd
from concourse.masks import make_identity

B, S, D, H, HD = 32, 256, 1024, 16, 64
NCORES = 8
BPC = B // NCORES  # 4 batches per core
F32 = mybir.dt.float32
BF16 = mybir.dt.bfloat16
NPBF = ml_dtypes.bfloat16
F8 = mybir.dt.float8e4
NPF8 = ml_dtypes.float8_e4m3
USCALE = 512.0  # fp8 range lift for the tiny folded U entries
DC = D // 128  # 8 chunks of the model dim
JC = S // 128  # 2 chunks of the sequence dim

_cache = {}


def _build():
    nc = bacc.Bacc("TRN2", target_bir_lowering=False, debug=False,
                   num_devices=NCORES)

    nv_ext = nc.declare_dram_parameter("nv", [BPC, 128, JC, D], BF16,
                                       isOutput=False)
    xt_ext = nc.declare_dram_parameter("xT", [BPC, 128, DC, 2 * S], F8,
                                       isOutput=False)
    u_ext = nc.declare_dram_parameter("U", [128, DC, 2 * H], F8,
                                      isOutput=False)
    wvt_ext = nc.declare_dram_parameter("WvT", [128, DC, DC, 128], BF16,
                                        isOutput=False)
    wot_ext = nc.declare_dram_parameter("WoT", [128, DC, DC, 128], BF16,
                                        isOutput=False)
    out_ext = nc.declare_dram_parameter("out", [128, DC, BPC], F32,
                                        isOutput=True)

    with tile.TileContext(nc) as tc, ExitStack() as ctx:
        wpool = ctx.enter_context(tc.tile_pool(name="w", bufs=1))
        xpool = ctx.enter_context(tc.tile_pool(name="x", bufs=4))
        smpool = ctx.enter_context(tc.tile_pool(name="sm", bufs=2))
        pspool = ctx.enter_context(tc.tile_pool(name="ps", bufs=2,
                                                space="PSUM"))

        # --- resident constants -------------------------------------------
        ones128 = wpool.tile([128, 1], BF16)
        nc.gpsimd.memset(ones128[:], 1.0)
        ones1 = wpool.tile([1, 128], BF16)
        nc.gpsimd.memset(ones1[:], 1.0)
        recips = wpool.tile([1, BPC * H], BF16)
        # double-buffered unnormalized-pT tiles.  Row j=0 (CLS, masked out)
        # needs no masking op: the xT layout plants a -448*sign(w_e) column
        # right before descT, so the e-term drives c[0,h] to ~-2e5 for every
        # head and exp underflows to exactly 0.
        ptus = [wpool.tile([128, JC, H], BF16, name=f"ptu{par}")
                for par in range(2)]
        u_sb = wpool.tile([128, DC, 2 * H], F8)
        # U goes on the gpsimd queue so it doesn't take a slot ahead of the
        # batch-0 activations on the sync queue.
        nc.gpsimd.dma_start(out=u_sb[:], in_=u_ext.ap())

        nvall = wpool.tile([128, DC, BPC * H], BF16)  # nvbarT, all batches
        ctx_sb = wpool.tile([128, DC, BPC], BF16)
        wvt_sb = wpool.tile([128, DC, DC, 128], BF16)
        wot_sb = wpool.tile([128, DC, DC, 128], BF16)

        # --- all input DMAs up-front, in consumption order ----------------
        xt_tiles, nv_tiles = [], []
        for b in range(BPC):
            xt_sb = xpool.tile([128, DC, 2 * S], F8, tag="xt")
            if b == 0:
                nc.sync.dma_start(out=xt_sb[:, 0:2], in_=xt_ext[b, :, 0:2])
                nc.sync.dma_start(out=xt_sb[:, 2:DC], in_=xt_ext[b, :, 2:DC])
            else:
                nc.sync.dma_start(out=xt_sb[:], in_=xt_ext[b])
            nv_sb = xpool.tile([128, JC, D], BF16, tag="nv")
            nc.sync.dma_start(out=nv_sb[:], in_=nv_ext[b])
            xt_tiles.append(xt_sb)
            nv_tiles.append(nv_sb)
        for sl in (slice(0, 4), slice(4, 8)):
            nc.sync.dma_start(out=wvt_sb[:, sl], in_=wvt_ext[:, sl])
        for sl in (slice(0, 4), slice(4, 8)):
            nc.sync.dma_start(out=wot_sb[:, sl], in_=wot_ext[:, sl])

        # --- batch loop, software-pipelined.  Logits come out j-major
        # (cT[j, h], j on partitions) by using xT as the DoubleRow
        # stationary operand — no p-transpose needed, and the softmax's
        # sum/recip/normalize moves OFF the per-batch critical path: nvbarT
        # uses unnormalized exp-weights and the 1/sum is applied to nvall
        # once, after the loop (PSUM f32 keeps the unnormalized sums exact).
        # The e-term needs e_j at row j from descT col j-1: the lhsT slice
        # shifts by one column; for jb=0 the slice starts at the nvT j=255
        # column, which corrupts only row j=0 — zeroed anyway (adj[:,0]=0).
        DR = mybir.MatmulPerfMode.DoubleRow

        def logits(b):
            xt_sb = xt_tiles[b]
            pscs = []
            for jb in range(JC):
                psc = pspool.tile([128, H], F32, tag="s", bufs=4,
                                  name=f"psc{b}_{jb}")
                jcol = jb * 128
                for c2 in range(DC // 2):
                    pair = slice(2 * c2, 2 * c2 + 2)
                    nc.tensor.matmul(psc[:],
                                     xt_sb[:, pair, jcol:jcol + 128],
                                     u_sb[:, pair, 0:H],
                                     start=(c2 == 0), stop=False,
                                     perf_mode=DR)
                for c2 in range(DC // 2):
                    pair = slice(2 * c2, 2 * c2 + 2)
                    nc.tensor.matmul(psc[:],
                                     xt_sb[:, pair,
                                           S + jcol:S + jcol + 128],
                                     u_sb[:, pair, H:2 * H],
                                     start=False, stop=(c2 == DC // 2 - 1),
                                     perf_mode=DR)
                pscs.append(psc)
            return pscs

        def tail(b, pscs):
            nv_sb = nv_tiles[b]
            ptu = ptus[b % 2]
            for jb in range(JC):
                nc.scalar.activation(ptu[:, jb, :], pscs[jb][:],
                                     mybir.ActivationFunctionType.Exp,
                                     scale=1.0 / USCALE)

            # nvbarT (unnormalized) for all 8 d-chunks.  Consecutive cm
            # groups alternate PSUM banks (a new accumulation group in the
            # bank the previous group used stalls the PE ~150ns).
            nb_ps = [pspool.tile([128, DC // 2, H], F32, tag=f"nb{par}",
                                 bufs=1, name=f"nb{par}_{b}")
                     for par in range(2)]
            for cm in range(DC):
                for jc in range(JC):
                    nc.tensor.matmul(nb_ps[cm % 2][:, cm // 2, :],
                                     nv_sb[:, jc, cm * 128:(cm + 1) * 128],
                                     ptu[:, jc, :],
                                     start=(jc == 0), stop=(jc == JC - 1))
            nvb = nvall[:, :, b * H:(b + 1) * H].rearrange(
                "p (c two) h -> p c two h", two=2)
            for par in range(2):
                nc.vector.tensor_copy(nvb[:, :, par, :], nb_ps[par][:])

            # sum_j p and its reciprocal — off the critical path
            s_ps = pspool.tile([1, H], F32, tag="s", bufs=4, name=f"sum{b}")
            for jb in range(JC):
                nc.tensor.matmul(s_ps[:], ones128[:], ptu[:, jb, :],
                                 start=(jb == 0), stop=(jb == JC - 1))
            with nc.allow_low_precision(reason="1/sum in bf16 (~0.4%) is "
                                        "far below the fp8 logits noise"):
                nc.vector.reciprocal(recips[0:1, b * H:(b + 1) * H], s_ps[:])

        pscs_live = logits(0)
        for b in range(BPC):
            pscs_next = logits(b + 1) if b + 1 < BPC else None
            tail(b, pscs_live)
            pscs_live = pscs_next

        # 1/sum broadcast across partitions via PE — consumed by the selects
        # (staged through SBUF: the select already reads Vbar from PSUM)
        r_ps = pspool.tile([128, BPC * H], F32, tag="nb0", bufs=1)
        nc.tensor.matmul(r_ps[:], ones1[:], recips[:], start=True, stop=True)
        r_sb = wpool.tile([128, BPC * H], F32)
        nc.vector.tensor_copy(r_sb[:], r_ps[:])

        # --- VbarT (unnormalized), blockdiag select fused with the 1/sum
        # scale, and the out-projection's ck-rounds interleaved two chunks
        # behind so OUT finishes right after the last Vbar chunk.
        # Consecutive cm groups alternate PSUM banks. ----------------------
        vb_ps = [pspool.tile([128, DC // 2, BPC * H], F32, tag=f"vb{par}",
                             bufs=1, name=f"vb{par}")
                 for par in range(2)]
        o_ps = [pspool.tile([128, DC // 2, BPC], F32, tag="s", bufs=4,
                            name=f"o{par}")
                for par in range(2)]

        def vbar(cm):
            for ck in range(DC):
                nc.tensor.matmul(vb_ps[cm % 2][:, cm // 2, :],
                                 wvt_sb[:, cm, ck, :],
                                 nvall[:, ck, :],
                                 start=(ck == 0), stop=(ck == DC - 1))

        def select(cm):
            for half in range(2):
                h = 2 * cm + half
                rows = slice(64 * half, 64 * half + 64)
                s_ap = vb_ps[cm % 2][rows, cm // 2, :].rearrange(
                    "p (b h) -> p b h", h=H)[:, :, h]
                r_ap = r_sb[rows, :].rearrange(
                    "p (b h) -> p b h", h=H)[:, :, h]
                nc.vector.tensor_mul(ctx_sb[rows, cm, :], s_ap, r_ap)

        for cm in range(DC):
            vbar(cm)
        for cm in range(DC):
            select(cm)
        for ec in range(DC):
            for ck in range(DC):
                nc.tensor.matmul(o_ps[ec % 2][:, ec // 2, :],
                                 wot_sb[:, ck, ec, :],
                                 ctx_sb[:, ck, :],
                                 start=(ck == 0), stop=(ck == DC - 1))

        o_sb = smpool.tile([128, DC, BPC], F32, tag="osb")
        for par in range(2):
            nc.vector.tensor_copy(
                o_sb[:].rearrange("p (e two) b -> p e two b", two=2)
                [:, :, par, :],
                o_ps[par][:])
        nc.sync.dma_start(out=out_ext.ap(), in_=o_sb[:])

    nc.compile()
    return nc


def _prep(desc, nv, Wk, Wv, Wo, attn_w):
    w_k = attn_w[HD:2 * HD]
    w_e = attn_w[2 * HD:]
    Uk = np.einsum('hmd,m->dh', Wk.reshape(H, HD, D), w_k)
    Ue = np.zeros((D, H), np.float32)
    for h in range(H):
        Ue[h * HD:(h + 1) * HD, h] = w_e
    U = np.concatenate([Uk, Ue], axis=1) * USCALE           # [D, 32]
    Up = np.ascontiguousarray(
        U.reshape(DC, 128, 2 * H).swapaxes(0, 1)).astype(NPF8)
    WvTp = np.ascontiguousarray(
        Wv.T.reshape(DC, 128, DC, 128).transpose(1, 2, 0, 3)).astype(NPBF)
    WoTp = np.ascontiguousarray(
        Wo.T.reshape(DC, 128, DC, 128).transpose(1, 0, 2, 3)).astype(NPBF)
    # nv natural, chunked over j: [B, 128, JC, D]
    nvp = np.ascontiguousarray(
        nv.reshape(B, JC, 128, D).swapaxes(1, 2)).astype(NPBF)
    # nv transposed, chunked over d: [B, 128, DC, S]
    nvTp = nv.transpose(0, 2, 1).reshape(B, DC, 128, S).swapaxes(1, 2)
    descTp = desc.transpose(0, 2, 1).reshape(B, DC, 128, S - 1).swapaxes(1, 2)
    # CLS kill column (sits at descT's j=0 slot): -K*sign(w_e) drives the
    # j=0 logit to ~-50 post-scale for every head (w_e is shared across
    # heads), so exp(j=0) ~ 1e-22 — dead, but safe for the Exp table
    kmag = 50.0 / max(np.abs(w_e).sum(), 1e-6)
    neg = np.zeros((D,), np.float32)
    for h in range(H):
        neg[h * HD:(h + 1) * HD] = -kmag * np.sign(w_e)
    negcol = np.broadcast_to(
        neg.reshape(DC, 128).T.reshape(1, 128, DC, 1), (B, 128, DC, 1))
    xTp = np.concatenate([nvTp, negcol, descTp], axis=3).astype(NPF8)
    return Up, WvTp, WoTp, nvp, xTp


def kernel(desc_embeddings, name_value_embeddings, Wq, bq, Wk, bk, Wv, bv,
           attn_w, attn_b, Wo, bo, _trace=False):
    desc = np.asarray(desc_embeddings, np.float32)
    nv = np.asarray(name_value_embeddings, np.float32)
    Up, WvTp, WoTp, nvp, xTp = _prep(
        desc, nv, np.asarray(Wk, np.float32), np.asarray(Wv, np.float32),
        np.asarray(Wo, np.float32), np.asarray(attn_w, np.float32))

    if "nc" not in _cache:
        _cache["nc"] = _build()
    nc = _cache["nc"]

    in_maps = []
    for c in range(NCORES):
        sl = slice(c * BPC, (c + 1) * BPC)
        in_maps.append({
            "nv": np.ascontiguousarray(nvp[sl]),
            "xT": np.ascontiguousarray(xTp[sl]),
            "U": Up, "WvT": WvTp, "WoT": WoTp,
        })
    res = run_bass_kernel_spmd(nc, in_maps, core_ids=list(range(NCORES)),
                               trace=_trace)
    out_rows = np.empty((B, D), np.float32)
    for c in range(NCORES):
        ot = np.asarray(res.results[c]["out"])  # [128, DC, BPC] = outT
        out_rows[c * BPC:(c + 1) * BPC] = ot.transpose(2, 1, 0).reshape(BPC, D)
    bo_eff = (np.asarray(bo, np.float32)
              + np.asarray(Wo, np.float32) @ np.asarray(bv, np.float32))
    out_rows += bo_eff[None, :]
    full = np.broadcast_to(out_rows[:, None, :], (B, S, D))
    if _trace:
        return np.ascontiguousarray(full), res
    return np.ascontiguousarray(full)


# revision 59
# speedup vs baseline: 1.1698x; 1.1189x over previous
"""AdaptiveGraphAttention Trainium2 kernel (8 NeuronCores, data-parallel).

Math: in the reference, logits[b,h,i,j] = a_q[b,h,i] + a_k[b,h,j] +
e_j[b,h,j]*adj[i,j] + attn_b with adj[:,0]=0, adj[:,1:]=1 — the mask and the
j-dependent terms are identical for every query row i, and the a_q/bias terms
are constant over j.  Softmax is shift-invariant, so the attention
distribution p[b,h,:] = softmax_{j>=1}(a_k + e_j) is the same for all i: the
attention matrix is rank-1 and the output is one row per batch, broadcast
over the 256 query positions.  bq/bk/attn_b cancel exactly; bv survives as
an additive constant (sum_j p_j = 1); bv and bo are folded on the host.

Per-head dots fold into small matrices:
  a_k[b,j,h] = nv[b,j,:] @ Uk[:,h],  Uk[d,h] = sum_m Wk[h*64+m, d] * w_k[m]
  e_j[b,j,h] = desc[b,j-1,:] @ Ue[:,h], Ue[h*64+m, h] = w_e(m) (else 0)

Device work per core (4 batches); fp8e4m3 logits path (inputs scaled by
USCALE to clear the fp8 subnormal floor, undone by the exp's scale), bf16
everywhere else, f32 PSUM accumulation:
  c[h,j]    = Uk.T @ nvT[:, j] + Ue.T @ descT[:, j-1]  (PE DoubleRow fp8)
  p[h,:]    = softmax_j(c)   (no max-subtraction: logits are O(1));
              the 1/sum normalization is folded into the p-transpose by
              multiplying with diag(recip) on the PE.
  nvbarT    = nv.T @ pT           [1024, 16] per batch      (PE, one PSUM
              tile for all 8 d-chunks -> single copy to SBUF)
  VbarT     = WvT.T @ nvbarT_all  [1024, 4*16] per d'-chunk (PE)
  ctxT      = blockdiag-select(VbarT)        [1024, 4]      (DVE)
  out       = ctxT.T @ WoT        [4, 1024]                 (PE) -> DMA
bv is folded into the host-side output bias (out += Wo @ bv + bo, exact
since sum_j p_j = 1).

Schedule: the kernel is HBM-DMA-bound (~8.4MB/core at ~358GB/s).  All DMAs
issue up-front on the sync queue in consumption order: per-batch activations
first (batch 0's xT split so the first DR matmul starts ~1us after the first
bytes land), then WvT in cm-halves, then WoT in output-column halves — the
Vbar and out stages chase the incoming weight stream instead of running as a
serial tail after it.

All DRAM inputs are host-prepermuted to [128, chunk, inner] so each DMA
partition row is one contiguous run (descriptor-count relief).
"""

import numpy as np
import ml_dtypes
from contextlib import ExitStack

import concourse.bass as bass
import concourse.mybir as mybir
import concourse.tile as tile
from concourse import bacc
from concourse.bass_utils import run_bass_kernel_spmd
from concourse.masks import make_identity

B, S, D, H, HD = 32, 256, 1024, 16, 64
NCORES = 8
BPC = B // NCORES  # 4 batches per core
F32 = mybir.dt.float32
BF16 = mybir.dt.bfloat16
NPBF = ml_dtypes.bfloat16
F8 = mybir.dt.float8e4
NPF8 = ml_dtypes.float8_e4m3
USCALE = 512.0  # fp8 range lift for the tiny folded U entries
DC = D // 128  # 8 chunks of the model dim
JC = S // 128  # 2 chunks of the sequence dim

_cache = {}


def _build():
    nc = bacc.Bacc("TRN2", target_bir_lowering=False, debug=False,
                   num_devices=NCORES)

    nv_ext = nc.declare_dram_parameter("nv", [BPC, 128, JC, D], BF16,
                                       isOutput=False)
    xt_ext = nc.declare_dram_parameter("xT", [BPC, 128, DC, 2 * S], F8,
                                       isOutput=False)
    u_ext = nc.declare_dram_parameter("U", [128, DC, 2 * H], F8,
                                      isOutput=False)
    wvt_ext = nc.declare_dram_parameter("WvT", [128, DC, DC, 128], BF16,
                                        isOutput=False)
    wot_ext = nc.declare_dram_parameter("WoT", [128, DC, DC, 128], BF16,
                                        isOutput=False)
    out_ext = nc.declare_dram_parameter("out", [128, DC, BPC], F32,
                                        isOutput=True)

    with tile.TileContext(nc) as tc, ExitStack() as ctx:
        wpool = ctx.enter_context(tc.tile_pool(name="w", bufs=1))
        xpool = ctx.enter_context(tc.tile_pool(name="x", bufs=4))
        smpool = ctx.enter_context(tc.tile_pool(name="sm", bufs=2))
        pspool = ctx.enter_context(tc.tile_pool(name="ps", bufs=2,
                                                space="PSUM"))

        # --- resident constants -------------------------------------------
        ones128 = wpool.tile([128, 1], BF16)
        nc.gpsimd.memset(ones128[:], 1.0)
        ones1 = wpool.tile([1, 128], BF16)
        nc.gpsimd.memset(ones1[:], 1.0)
        recips = wpool.tile([1, BPC * H], BF16)
        # double-buffered unnormalized-pT tiles.  Row j=0 (CLS, masked out)
        # needs no masking op: the xT layout plants a -448*sign(w_e) column
        # right before descT, so the e-term drives c[0,h] to ~-2e5 for every
        # head and exp underflows to exactly 0.
        ptus = [wpool.tile([128, JC, H], BF16, name=f"ptu{par}")
                for par in range(2)]
        u_sb = wpool.tile([128, DC, 2 * H], F8)
        # U goes on the gpsimd queue so it doesn't take a slot ahead of the
        # batch-0 activations on the sync queue.
        nc.gpsimd.dma_start(out=u_sb[:], in_=u_ext.ap())

        nvall = wpool.tile([128, DC, BPC * H], BF16)  # nvbarT, all batches
        ctx_sb = wpool.tile([128, DC, BPC], BF16)
        wvt_sb = wpool.tile([128, DC, DC, 128], BF16)
        wot_sb = wpool.tile([128, DC, DC, 128], BF16)

        # --- all input DMAs up-front, in consumption order ----------------
        xt_tiles, nv_tiles = [], []
        for b in range(BPC):
            xt_sb = xpool.tile([128, DC, 2 * S], F8, tag="xt")
            if b == 0:
                nc.sync.dma_start(out=xt_sb[:, 0:2], in_=xt_ext[b, :, 0:2])
                nc.sync.dma_start(out=xt_sb[:, 2:DC], in_=xt_ext[b, :, 2:DC])
            else:
                nc.sync.dma_start(out=xt_sb[:], in_=xt_ext[b])
            nv_sb = xpool.tile([128, JC, D], BF16, tag="nv")
            nc.sync.dma_start(out=nv_sb[:], in_=nv_ext[b])
            xt_tiles.append(xt_sb)
            nv_tiles.append(nv_sb)
        for sl in (slice(0, 4), slice(4, 8)):
            nc.sync.dma_start(out=wvt_sb[:, sl], in_=wvt_ext[:, sl])
        for sl in (slice(0, 4), slice(4, 8)):
            nc.sync.dma_start(out=wot_sb[:, sl], in_=wot_ext[:, sl])

        # --- batch loop, software-pipelined.  Logits come out j-major
        # (cT[j, h], j on partitions) by using xT as the DoubleRow
        # stationary operand — no p-transpose needed, and the softmax's
        # sum/recip/normalize moves OFF the per-batch critical path: nvbarT
        # uses unnormalized exp-weights and the 1/sum is applied to nvall
        # once, after the loop (PSUM f32 keeps the unnormalized sums exact).
        # The e-term needs e_j at row j from descT col j-1: the lhsT slice
        # shifts by one column; for jb=0 the slice starts at the nvT j=255
        # column, which corrupts only row j=0 — zeroed anyway (adj[:,0]=0).
        DR = mybir.MatmulPerfMode.DoubleRow

        def logits(b):
            xt_sb = xt_tiles[b]
            pscs = []
            for jb in range(JC):
                psc = pspool.tile([128, H], F32, tag="s", bufs=4,
                                  name=f"psc{b}_{jb}")
                jcol = jb * 128
                for c2 in range(DC // 2):
                    pair = slice(2 * c2, 2 * c2 + 2)
                    nc.tensor.matmul(psc[:],
                                     xt_sb[:, pair, jcol:jcol + 128],
                                     u_sb[:, pair, 0:H],
                                     start=(c2 == 0), stop=False,
                                     perf_mode=DR)
                for c2 in range(DC // 2):
                    pair = slice(2 * c2, 2 * c2 + 2)
                    nc.tensor.matmul(psc[:],
                                     xt_sb[:, pair,
                                           S + jcol:S + jcol + 128],
                                     u_sb[:, pair, H:2 * H],
                                     start=False, stop=(c2 == DC // 2 - 1),
                                     perf_mode=DR)
                pscs.append(psc)
            return pscs

        def tail(b, pscs):
            nv_sb = nv_tiles[b]
            ptu = ptus[b % 2]
            for jb in range(JC):
                nc.scalar.activation(ptu[:, jb, :], pscs[jb][:],
                                     mybir.ActivationFunctionType.Exp,
                                     scale=1.0 / USCALE)

            # nvbarT (unnormalized) for all 8 d-chunks.  Consecutive cm
            # groups alternate PSUM banks (a new accumulation group in the
            # bank the previous group used stalls the PE ~150ns).
            nb_ps = [pspool.tile([128, DC // 2, H], F32, tag=f"nb{par}",
                                 bufs=1, name=f"nb{par}_{b}")
                     for par in range(2)]
            for cm in range(DC):
                for jc in range(JC):
                    nc.tensor.matmul(nb_ps[cm % 2][:, cm // 2, :],
                                     nv_sb[:, jc, cm * 128:(cm + 1) * 128],
                                     ptu[:, jc, :],
                                     start=(jc == 0), stop=(jc == JC - 1))
            nvb = nvall[:, :, b * H:(b + 1) * H].rearrange(
                "p (c two) h -> p c two h", two=2)
            for par in range(2):
                nc.vector.tensor_copy(nvb[:, :, par, :], nb_ps[par][:])

            # sum_j p and its reciprocal — off the critical path
            s_ps = pspool.tile([1, H], F32, tag="s", bufs=4, name=f"sum{b}")
            for jb in range(JC):
                nc.tensor.matmul(s_ps[:], ones128[:], ptu[:, jb, :],
                                 start=(jb == 0), stop=(jb == JC - 1))
            with nc.allow_low_precision(reason="1/sum in bf16 (~0.4%) is "
                                        "far below the fp8 logits noise"):
                nc.vector.reciprocal(recips[0:1, b * H:(b + 1) * H], s_ps[:])

        pscs_live = logits(0)
        for b in range(BPC):
            pscs_next = logits(b + 1) if b + 1 < BPC else None
            tail(b, pscs_live)
            pscs_live = pscs_next

        # 1/sum broadcast across partitions via PE — consumed by the selects
        # (staged through SBUF: the select already reads Vbar from PSUM)
        r_ps = pspool.tile([128, BPC * H], F32, tag="nb0", bufs=1)
        nc.tensor.matmul(r_ps[:], ones1[:], recips[:], start=True, stop=True)
        r_sb = wpool.tile([128, BPC * H], F32)
        nc.vector.tensor_copy(r_sb[:], r_ps[:])

        # --- VbarT (unnormalized), blockdiag select fused with the 1/sum
        # scale, and the out-projection's ck-rounds interleaved two chunks
        # behind so OUT finishes right after the last Vbar chunk.
        # Consecutive cm groups alternate PSUM banks. ----------------------
        vb_ps = [pspool.tile([128, DC // 2, BPC * H], F32, tag=f"vb{par}",
                             bufs=1, name=f"vb{par}")
                 for par in range(2)]
        o_ps = [pspool.tile([128, DC // 2, BPC], F32, tag="s", bufs=4,
                            name=f"o{par}")
                for par in range(2)]

        def vbar(cm):
            for ck in range(DC):
                nc.tensor.matmul(vb_ps[cm % 2][:, cm // 2, :],
                                 wvt_sb[:, cm, ck, :],
                                 nvall[:, ck, :],
                                 start=(ck == 0), stop=(ck == DC - 1))

        def select(cm):
            for half in range(2):
                h = 2 * cm + half
                rows = slice(64 * half, 64 * half + 64)
                s_ap = vb_ps[cm % 2][rows, cm // 2, :].rearrange(
                    "p (b h) -> p b h", h=H)[:, :, h]
                r_ap = r_sb[rows, :].rearrange(
                    "p (b h) -> p b h", h=H)[:, :, h]
                nc.vector.tensor_mul(ctx_sb[rows, cm, :], s_ap, r_ap)

        for cm in range(DC):
            vbar(cm)
        for cm in range(DC):
            select(cm)
        for half in range(2):
            for ec in range(4 * half, 4 * half + 4):
                for ck in range(DC):
                    nc.tensor.matmul(o_ps[ec % 2][:, ec // 2, :],
                                     wot_sb[:, ec, ck, :],
                                     ctx_sb[:, ck, :],
                                     start=(ck == 0), stop=(ck == DC - 1))
            ecs = slice(4 * half, 4 * half + 4)
            o_sb = smpool.tile([128, 4, BPC], F32, tag="osb",
                               name=f"osb{half}")
            for par in range(2):
                nc.vector.tensor_copy(
                    o_sb[:].rearrange("p (e two) b -> p e two b", two=2)
                    [:, :, par, :],
                    o_ps[par][:, 2 * half:2 * half + 2, :])
            nc.sync.dma_start(out=out_ext[:, ecs], in_=o_sb[:])

    nc.compile()
    return nc


def _prep(desc, nv, Wk, Wv, Wo, attn_w):
    w_k = attn_w[HD:2 * HD]
    w_e = attn_w[2 * HD:]
    Uk = np.einsum('hmd,m->dh', Wk.reshape(H, HD, D), w_k)
    Ue = np.zeros((D, H), np.float32)
    for h in range(H):
        Ue[h * HD:(h + 1) * HD, h] = w_e
    U = np.concatenate([Uk, Ue], axis=1) * USCALE           # [D, 32]
    Up = np.ascontiguousarray(
        U.reshape(DC, 128, 2 * H).swapaxes(0, 1)).astype(NPF8)
    WvTp = np.ascontiguousarray(
        Wv.T.reshape(DC, 128, DC, 128).transpose(1, 2, 0, 3)).astype(NPBF)
    WoTp = np.ascontiguousarray(
        Wo.T.reshape(DC, 128, DC, 128).transpose(1, 2, 0, 3)).astype(NPBF)
    # nv natural, chunked over j: [B, 128, JC, D]
    nvp = np.ascontiguousarray(
        nv.reshape(B, JC, 128, D).swapaxes(1, 2)).astype(NPBF)
    # nv transposed, chunked over d: [B, 128, DC, S]
    nvTp = nv.transpose(0, 2, 1).reshape(B, DC, 128, S).swapaxes(1, 2)
    descTp = desc.transpose(0, 2, 1).reshape(B, DC, 128, S - 1).swapaxes(1, 2)
    # CLS kill column (sits at descT's j=0 slot): -K*sign(w_e) drives the
    # j=0 logit to ~-50 post-scale for every head (w_e is shared across
    # heads), so exp(j=0) ~ 1e-22 — dead, but safe for the Exp table
    kmag = 50.0 / max(np.abs(w_e).sum(), 1e-6)
    neg = np.zeros((D,), np.float32)
    for h in range(H):
        neg[h * HD:(h + 1) * HD] = -kmag * np.sign(w_e)
    negcol = np.broadcast_to(
        neg.reshape(DC, 128).T.reshape(1, 128, DC, 1), (B, 128, DC, 1))
    xTp = np.concatenate([nvTp, negcol, descTp], axis=3).astype(NPF8)
    return Up, WvTp, WoTp, nvp, xTp


def kernel(desc_embeddings, name_value_embeddings, Wq, bq, Wk, bk, Wv, bv,
           attn_w, attn_b, Wo, bo, _trace=False):
    desc = np.asarray(desc_embeddings, np.float32)
    nv = np.asarray(name_value_embeddings, np.float32)
    Up, WvTp, WoTp, nvp, xTp = _prep(
        desc, nv, np.asarray(Wk, np.float32), np.asarray(Wv, np.float32),
        np.asarray(Wo, np.float32), np.asarray(attn_w, np.float32))

    if "nc" not in _cache:
        _cache["nc"] = _build()
    nc = _cache["nc"]

    in_maps = []
    for c in range(NCORES):
        sl = slice(c * BPC, (c + 1) * BPC)
        in_maps.append({
            "nv": np.ascontiguousarray(nvp[sl]),
            "xT": np.ascontiguousarray(xTp[sl]),
            "U": Up, "WvT": WvTp, "WoT": WoTp,
        })
    res = run_bass_kernel_spmd(nc, in_maps, core_ids=list(range(NCORES)),
                               trace=_trace)
    out_rows = np.empty((B, D), np.float32)
    for c in range(NCORES):
        ot = np.asarray(res.results[c]["out"])  # [128, DC, BPC] = outT
        out_rows[c * BPC:(c + 1) * BPC] = ot.transpose(2, 1, 0).reshape(BPC, D)
    bo_eff = (np.asarray(bo, np.float32)
              + np.asarray(Wo, np.float32) @ np.asarray(bv, np.float32))
    out_rows += bo_eff[None, :]
    full = np.broadcast_to(out_rows[:, None, :], (B, S, D))
    if _trace:
        return np.ascontiguousarray(full), res
    return np.ascontiguousarray(full)


# revision 61
# speedup vs baseline: 1.1797x; 1.0085x over previous
"""AdaptiveGraphAttention Trainium2 kernel (8 NeuronCores, data-parallel).

Math: in the reference, logits[b,h,i,j] = a_q[b,h,i] + a_k[b,h,j] +
e_j[b,h,j]*adj[i,j] + attn_b with adj[:,0]=0, adj[:,1:]=1 — the mask and the
j-dependent terms are identical for every query row i, and the a_q/bias terms
are constant over j.  Softmax is shift-invariant, so the attention
distribution p[b,h,:] = softmax_{j>=1}(a_k + e_j) is the same for all i: the
attention matrix is rank-1 and the output is one row per batch, broadcast
over the 256 query positions.  bq/bk/attn_b cancel exactly; bv survives as
an additive constant (sum_j p_j = 1); bv and bo are folded on the host.

Per-head dots fold into small matrices:
  a_k[b,j,h] = nv[b,j,:] @ Uk[:,h],  Uk[d,h] = sum_m Wk[h*64+m, d] * w_k[m]
  e_j[b,j,h] = desc[b,j-1,:] @ Ue[:,h], Ue[h*64+m, h] = w_e(m) (else 0)

Device work per core (4 batches); fp8e4m3 logits path (inputs scaled by
USCALE to clear the fp8 subnormal floor, undone by the exp's scale), bf16
everywhere else, f32 PSUM accumulation:
  c[h,j]    = Uk.T @ nvT[:, j] + Ue.T @ descT[:, j-1]  (PE DoubleRow fp8)
  p[h,:]    = softmax_j(c)   (no max-subtraction: logits are O(1));
              the 1/sum normalization is folded into the p-transpose by
              multiplying with diag(recip) on the PE.
  nvbarT    = nv.T @ pT           [1024, 16] per batch      (PE, one PSUM
              tile for all 8 d-chunks -> single copy to SBUF)
  VbarT     = WvT.T @ nvbarT_all  [1024, 4*16] per d'-chunk (PE)
  ctxT      = blockdiag-select(VbarT)        [1024, 4]      (DVE)
  out       = ctxT.T @ WoT        [4, 1024]                 (PE) -> DMA
bv is folded into the host-side output bias (out += Wo @ bv + bo, exact
since sum_j p_j = 1).

Schedule: the kernel is HBM-DMA-bound (~8.4MB/core at ~358GB/s).  All DMAs
issue up-front on the sync queue in consumption order: per-batch activations
first (batch 0's xT split so the first DR matmul starts ~1us after the first
bytes land), then WvT in cm-halves, then WoT in output-column halves — the
Vbar and out stages chase the incoming weight stream instead of running as a
serial tail after it.

All DRAM inputs are host-prepermuted to [128, chunk, inner] so each DMA
partition row is one contiguous run (descriptor-count relief).
"""

import numpy as np
import ml_dtypes
from contextlib import ExitStack

import concourse.bass as bass
import concourse.mybir as mybir
import concourse.tile as tile
from concourse import bacc
from concourse.bass_utils import run_bass_kernel_spmd
from concourse.masks import make_identity

B, S, D, H, HD = 32, 256, 1024, 16, 64
NCORES = 8
BPC = B // NCORES  # 4 batches per core
F32 = mybir.dt.float32
BF16 = mybir.dt.bfloat16
NPBF = ml_dtypes.bfloat16
F8 = mybir.dt.float8e4
NPF8 = ml_dtypes.float8_e4m3
USCALE = 512.0  # fp8 range lift for the tiny folded U entries
DC = D // 128  # 8 chunks of the model dim
JC = S // 128  # 2 chunks of the sequence dim

_cache = {}


def _build():
    nc = bacc.Bacc("TRN2", target_bir_lowering=False, debug=False,
                   num_devices=NCORES)

    nv_ext = nc.declare_dram_parameter("nv", [BPC, 128, JC, D], BF16,
                                       isOutput=False)
    xt_ext = nc.declare_dram_parameter("xT", [BPC, 128, DC, 2 * S], F8,
                                       isOutput=False)
    u_ext = nc.declare_dram_parameter("U", [128, DC, 2 * H], F8,
                                      isOutput=False)
    wvt_ext = nc.declare_dram_parameter("WvT", [128, DC, DC, 128], BF16,
                                        isOutput=False)
    wot_ext = nc.declare_dram_parameter("WoT", [128, DC, DC, 128], BF16,
                                        isOutput=False)
    out_ext = nc.declare_dram_parameter("out", [128, DC, BPC], F32,
                                        isOutput=True)

    with tile.TileContext(nc) as tc, ExitStack() as ctx:
        wpool = ctx.enter_context(tc.tile_pool(name="w", bufs=1))
        xpool = ctx.enter_context(tc.tile_pool(name="x", bufs=4))
        smpool = ctx.enter_context(tc.tile_pool(name="sm", bufs=2))
        pspool = ctx.enter_context(tc.tile_pool(name="ps", bufs=2,
                                                space="PSUM"))

        # --- resident constants -------------------------------------------
        ones128 = wpool.tile([128, 1], BF16)
        nc.gpsimd.memset(ones128[:], 1.0)
        ones1 = wpool.tile([1, 128], BF16)
        nc.gpsimd.memset(ones1[:], 1.0)
        recips = wpool.tile([1, BPC * H], BF16)
        # double-buffered unnormalized-pT tiles.  Row j=0 (CLS, masked out)
        # needs no masking op: the xT layout plants a -448*sign(w_e) column
        # right before descT, so the e-term drives c[0,h] to ~-2e5 for every
        # head and exp underflows to exactly 0.
        ptus = [wpool.tile([128, JC, H], BF16, name=f"ptu{par}")
                for par in range(2)]
        u_sb = wpool.tile([128, DC, 2 * H], F8)
        # U goes on the gpsimd queue so it doesn't take a slot ahead of the
        # batch-0 activations on the sync queue.
        nc.gpsimd.dma_start(out=u_sb[:], in_=u_ext.ap())

        nvall = wpool.tile([128, DC, BPC * H], BF16)  # nvbarT, all batches
        ctx_sb = wpool.tile([128, DC, BPC], BF16)
        wvt_sb = wpool.tile([128, DC, DC, 128], BF16)
        wot_sb = wpool.tile([128, DC, DC, 128], BF16)

        # --- all input DMAs up-front, in consumption order ----------------
        xt_tiles, nv_tiles = [], []
        for b in range(BPC):
            xt_sb = xpool.tile([128, DC, 2 * S], F8, tag="xt")
            if b == 0:
                nc.sync.dma_start(out=xt_sb[:, 0:2], in_=xt_ext[b, :, 0:2])
                nc.sync.dma_start(out=xt_sb[:, 2:DC], in_=xt_ext[b, :, 2:DC])
            else:
                nc.sync.dma_start(out=xt_sb[:], in_=xt_ext[b])
            nv_sb = xpool.tile([128, JC, D], BF16, tag="nv")
            nc.sync.dma_start(out=nv_sb[:], in_=nv_ext[b])
            xt_tiles.append(xt_sb)
            nv_tiles.append(nv_sb)
        for sl in (slice(0, 4), slice(4, 8)):
            nc.sync.dma_start(out=wvt_sb[:, sl], in_=wvt_ext[:, sl])
        for sl in (slice(0, 4), slice(4, 8)):
            nc.sync.dma_start(out=wot_sb[:, sl], in_=wot_ext[:, sl])

        # --- batch loop, software-pipelined.  Logits come out j-major
        # (cT[j, h], j on partitions) by using xT as the DoubleRow
        # stationary operand — no p-transpose needed, and the softmax's
        # sum/recip/normalize moves OFF the per-batch critical path: nvbarT
        # uses unnormalized exp-weights and the 1/sum is applied to nvall
        # once, after the loop (PSUM f32 keeps the unnormalized sums exact).
        # The e-term needs e_j at row j from descT col j-1: the lhsT slice
        # shifts by one column; for jb=0 the slice starts at the nvT j=255
        # column, which corrupts only row j=0 — zeroed anyway (adj[:,0]=0).
        DR = mybir.MatmulPerfMode.DoubleRow

        def logits(b):
            xt_sb = xt_tiles[b]
            pscs = []
            for jb in range(JC):
                psc = pspool.tile([128, H], F32, tag="s", bufs=4,
                                  name=f"psc{b}_{jb}")
                jcol = jb * 128
                for c2 in range(DC // 2):
                    pair = slice(2 * c2, 2 * c2 + 2)
                    nc.tensor.matmul(psc[:],
                                     xt_sb[:, pair, jcol:jcol + 128],
                                     u_sb[:, pair, 0:H],
                                     start=(c2 == 0), stop=False,
                                     perf_mode=DR)
                for c2 in range(DC // 2):
                    pair = slice(2 * c2, 2 * c2 + 2)
                    nc.tensor.matmul(psc[:],
                                     xt_sb[:, pair,
                                           S + jcol:S + jcol + 128],
                                     u_sb[:, pair, H:2 * H],
                                     start=False, stop=(c2 == DC // 2 - 1),
                                     perf_mode=DR)
                pscs.append(psc)
            return pscs

        def tail(b, pscs):
            nv_sb = nv_tiles[b]
            ptu = ptus[b % 2]
            for jb in range(JC):
                nc.scalar.activation(ptu[:, jb, :], pscs[jb][:],
                                     mybir.ActivationFunctionType.Exp,
                                     scale=1.0 / USCALE)

            # nvbarT (unnormalized) for all 8 d-chunks.  Consecutive cm
            # groups alternate PSUM banks (a new accumulation group in the
            # bank the previous group used stalls the PE ~150ns).
            nb_ps = [pspool.tile([128, DC // 2, H], F32, tag=f"nb{par}",
                                 bufs=1, name=f"nb{par}_{b}")
                     for par in range(2)]
            for cm in range(DC):
                for jc in range(JC):
                    nc.tensor.matmul(nb_ps[cm % 2][:, cm // 2, :],
                                     nv_sb[:, jc, cm * 128:(cm + 1) * 128],
                                     ptu[:, jc, :],
                                     start=(jc == 0), stop=(jc == JC - 1))
            nvb = nvall[:, :, b * H:(b + 1) * H].rearrange(
                "p (c two) h -> p c two h", two=2)
            for par in range(2):
                nc.vector.tensor_copy(nvb[:, :, par, :], nb_ps[par][:])

            # sum_j p and its reciprocal — off the critical path
            s_ps = pspool.tile([1, H], F32, tag="s", bufs=4, name=f"sum{b}")
            for jb in range(JC):
                nc.tensor.matmul(s_ps[:], ones128[:], ptu[:, jb, :],
                                 start=(jb == 0), stop=(jb == JC - 1))
            with nc.allow_low_precision(reason="1/sum in bf16 (~0.4%) is "
                                        "far below the fp8 logits noise"):
                nc.vector.reciprocal(recips[0:1, b * H:(b + 1) * H], s_ps[:])

        pscs_live = logits(0)
        for b in range(BPC):
            pscs_next = logits(b + 1) if b + 1 < BPC else None
            tail(b, pscs_live)
            pscs_live = pscs_next

        # 1/sum broadcast across partitions via PE — consumed by the selects
        # (staged through SBUF: the select already reads Vbar from PSUM)
        r_ps = pspool.tile([128, BPC * H], F32, tag="nb0", bufs=1)
        nc.tensor.matmul(r_ps[:], ones1[:], recips[:], start=True, stop=True)
        r_sb = wpool.tile([128, BPC * H], F32)
        nc.vector.tensor_copy(r_sb[:], r_ps[:])

        # --- VbarT (unnormalized), blockdiag select fused with the 1/sum
        # scale, and the out-projection's ck-rounds interleaved two chunks
        # behind so OUT finishes right after the last Vbar chunk.
        # Consecutive cm groups alternate PSUM banks. ----------------------
        vb_ps = [pspool.tile([128, DC // 2, BPC * H], F32, tag=f"vb{par}",
                             bufs=1, name=f"vb{par}")
                 for par in range(2)]
        o_ps = [pspool.tile([128, DC // 2, BPC], F32, tag="s", bufs=4,
                            name=f"o{par}")
                for par in range(2)]

        def vbar(cm):
            for ck in range(DC):
                nc.tensor.matmul(vb_ps[cm % 2][:, cm // 2, :],
                                 wvt_sb[:, cm, ck, :],
                                 nvall[:, ck, :],
                                 start=(ck == 0), stop=(ck == DC - 1))

        def select(cm):
            for half in range(2):
                h = 2 * cm + half
                rows = slice(64 * half, 64 * half + 64)
                s_ap = vb_ps[cm % 2][rows, cm // 2, :].rearrange(
                    "p (b h) -> p b h", h=H)[:, :, h]
                r_ap = r_sb[rows, :].rearrange(
                    "p (b h) -> p b h", h=H)[:, :, h]
                nc.vector.tensor_mul(ctx_sb[rows, cm, :], s_ap, r_ap)

        for cm in range(DC):
            vbar(cm)
        for cm in range(DC):
            select(cm)
        for half in range(2):
            for ec in range(4 * half, 4 * half + 4):
                for ck in range(DC):
                    nc.tensor.matmul(o_ps[ec % 2][:, ec // 2, :],
                                     wot_sb[:, ec, ck, :],
                                     ctx_sb[:, ck, :],
                                     start=(ck == 0), stop=(ck == DC - 1))
            ecs = slice(4 * half, 4 * half + 4)
            o_sb = smpool.tile([128, 4, BPC], F32, tag="osb",
                               name=f"osb{half}")
            for par in range(2):
                nc.vector.tensor_copy(
                    o_sb[:].rearrange("p (e two) b -> p e two b", two=2)
                    [:, :, par, :],
                    o_ps[par][:, 2 * half:2 * half + 2, :])
            nc.sync.dma_start(out=out_ext[:, ecs], in_=o_sb[:])

    nc.compile()
    return nc


def _prep(desc, nv, Wk, Wv, Wo, attn_w):
    w_k = attn_w[HD:2 * HD]
    w_e = attn_w[2 * HD:]
    Uk = np.einsum('hmd,m->dh', Wk.reshape(H, HD, D), w_k)
    Ue = np.zeros((D, H), np.float32)
    for h in range(H):
        Ue[h * HD:(h + 1) * HD, h] = w_e
    U = np.concatenate([Uk, Ue], axis=1) * USCALE           # [D, 32]
    Up = np.ascontiguousarray(
        U.reshape(DC, 128, 2 * H).swapaxes(0, 1)).astype(NPF8)
    WvTp = np.ascontiguousarray(
        Wv.T.reshape(DC, 128, DC, 128).transpose(1, 2, 0, 3)).astype(NPBF)
    WoTp = np.ascontiguousarray(
        Wo.T.reshape(DC, 128, DC, 128).transpose(1, 2, 0, 3)).astype(NPBF)
    # nv natural, chunked over j: [B, 128, JC, D]
    nvp = np.ascontiguousarray(
        nv.reshape(B, JC, 128, D).swapaxes(1, 2)).astype(NPBF)
    # nv transposed, chunked over d: [B, 128, DC, S]
    nvTp = nv.transpose(0, 2, 1).reshape(B, DC, 128, S).swapaxes(1, 2)
    descTp = desc.transpose(0, 2, 1).reshape(B, DC, 128, S - 1).swapaxes(1, 2)
    # CLS kill column (sits at descT's j=0 slot): -K*sign(w_e) drives the
    # j=0 logit to ~-50 post-scale for every head (w_e is shared across
    # heads), so exp(j=0) ~ 1e-22 — dead, but safe for the Exp table
    kmag = 50.0 / max(np.abs(w_e).sum(), 1e-6)
    neg = np.zeros((D,), np.float32)
    for h in range(H):
        neg[h * HD:(h + 1) * HD] = -kmag * np.sign(w_e)
    negcol = np.broadcast_to(
        neg.reshape(DC, 128).T.reshape(1, 128, DC, 1), (B, 128, DC, 1))
    xTp = np.concatenate([nvTp, negcol, descTp], axis=3).astype(NPF8)
    return Up, WvTp, WoTp, nvp, xTp


def kernel(desc_embeddings, name_value_embeddings, Wq, bq, Wk, bk, Wv, bv,
           attn_w, attn_b, Wo, bo, _trace=False):
    desc = np.asarray(desc_embeddings, np.float32)
    nv = np.asarray(name_value_embeddings, np.float32)
    Up, WvTp, WoTp, nvp, xTp = _prep(
        desc, nv, np.asarray(Wk, np.float32), np.asarray(Wv, np.float32),
        np.asarray(Wo, np.float32), np.asarray(attn_w, np.float32))

    if "nc" not in _cache:
        _cache["nc"] = _build()
    nc = _cache["nc"]

    in_maps = []
    for c in range(NCORES):
        sl = slice(c * BPC, (c + 1) * BPC)
        in_maps.append({
            "nv": np.ascontiguousarray(nvp[sl]),
            "xT": np.ascontiguousarray(xTp[sl]),
            "U": Up, "WvT": WvTp, "WoT": WoTp,
        })
    res = run_bass_kernel_spmd(nc, in_maps, core_ids=list(range(NCORES)),
                               trace=_trace)
    out_rows = np.empty((B, D), np.float32)
    for c in range(NCORES):
        ot = np.asarray(res.results[c]["out"])  # [128, DC, BPC] = outT
        out_rows[c * BPC:(c + 1) * BPC] = ot.transpose(2, 1, 0).reshape(BPC, D)
    bo_eff = (np.asarray(bo, np.float32)
              + np.asarray(Wo, np.float32) @ np.asarray(bv, np.float32))
    out_rows += bo_eff[None, :]
    full = np.broadcast_to(out_rows[:, None, :], (B, S, D))
    if _trace:
        return np.ascontiguousarray(full), res
    return np.ascontiguousarray(full)
